# revision 3
# baseline (speedup 1.0000x reference)
"""Trainium2 Bass kernel for a SiamRPN-style depthwise-xcorr head.

Computation (per batch sample):
  k = relu(bn(conv3x3(kernel, wk)))      # (256,7,7)  -> (256,5,5)
  s = relu(bn(conv3x3(search, ws)))      # (256,31,31)-> (256,29,29)
  f = depthwise_xcorr(s, k)              # (256,25,25)
  f = relu(bn(conv1x1(f, w1)))
  out = conv1x1(f, w2) + b2              # (256,25,25)

Sharding: data-parallel over batch, 8 samples per NeuronCore x 8 cores.
BN (eval mode) is folded into the conv weights on the host; the per-channel
shift is applied as the ScalarE activation bias during PSUM eviction.

All matmuls run in bf16 (1 cycle/row at any moving width, so no even-width
padding is needed anywhere); PSUM accumulation stays fp32.  The depthwise
xcorr's 25 taps per (sample, channel-half) chain are distributed across all
four compute engines:
  - PE taps: diagonal matmuls (diag built on ScalarE) accumulated in PSUM
  - DVE taps: bf16 product (tensor_scalar, 4x DVE mode) + bf16 add
    (tensor_tensor, 2x mode) into accD
  - Act taps: product on ScalarE (Copy with per-partition scale), add on DVE
  - Pool taps: product on DVE, add on GpSimd (tensor_tensor)
The two accumulators are merged on DVE and folded into the PE taps' PSUM
with an identity matmul; ScalarE evicts the result.  The scalar-engine
product stream is interleaved with the next sample's conv PSUM evictions
at emission time so neither blocks the other in the in-order Act queue.
"""

import sys

if "/opt/trn_rl_repo" not in sys.path:
    sys.path.insert(0, "/opt/trn_rl_repo")

import numpy as np
import ml_dtypes

import concourse.bacc as bacc
import concourse.mybir as mybir
import concourse.tile as tile
from concourse.bass_utils import run_bass_kernel_spmd

EPS = 1e-5
B, CIN, H, SK, SS, COUT = 64, 256, 256, 7, 31, 256
NCORES = 8
NB = B // NCORES            # samples per core
OS = SS - 2                 # 29: search conv output
OK = SK - 2                 # 5: kernel conv output
OX = OS - OK + 1            # 25: xcorr output
NPIX = OX * OX              # 625
NT = OK * OK                # 25 taps

F32 = mybir.dt.float32
BF16 = mybir.dt.bfloat16
BF16NP = ml_dtypes.bfloat16

# output-row chunks: each accumulated matmul's dst must sit inside one
# 512-f32 PSUM bank
S_CHUNKS = [(0, 15), (15, 14)]   # 15*29=435, 14*29=406
X_CHUNKS = [(0, 13), (13, 12)]   # 13*25=325, 12*25=300

# xcorr tap split per (sample, channel-half) chain: counts for
# (PE, DVE-pair, Act-product+DVE-add, Pool-add) taps, summing to 25.
SPLIT_MID = (3, 9, 6, 7)
SPLIT_LAST = (12, 7, 3, 3)      # last sample: nothing hides the chains

_CACHED = {}


def _build_nc():
    nc = bacc.Bacc("TRN2", target_bir_lowering=False, debug=False,
                   num_devices=NCORES)

    xs_d = nc.dram_tensor("xs", [NB, CIN, SS, SS], BF16, kind="ExternalInput")
    xk_d = nc.dram_tensor("xk", [2, 128, NB * SK * SK], BF16,
                          kind="ExternalInput")
    ws_d = nc.dram_tensor("ws", [128, 9, 2, 256], BF16, kind="ExternalInput")
    wk_d = nc.dram_tensor("wk", [128, 9, 2, 256], BF16, kind="ExternalInput")
    w1_d = nc.dram_tensor("w1", [128, 2, 256], BF16, kind="ExternalInput")
    w2_d = nc.dram_tensor("w2", [128, 2, 256], BF16, kind="ExternalInput")
    bb_d = nc.dram_tensor("bb", [128, 8], F32, kind="ExternalInput")
    id_d = nc.dram_tensor("id128", [128, 128], BF16, kind="ExternalInput")
    y_d = nc.dram_tensor("y", [NB, COUT, NPIX], F32, kind="ExternalOutput")

    RELU = mybir.ActivationFunctionType.Relu
    IDENT = mybir.ActivationFunctionType.Identity
    COPY = mybir.ActivationFunctionType.Copy
    MULT = mybir.AluOpType.mult
    ADD = mybir.AluOpType.add

    with tile.TileContext(nc) as tc:
        with (
            tc.tile_pool(name="wpool", bufs=1) as wpool,
            tc.tile_pool(name="xin", bufs=4) as xin,
            tc.tile_pool(name="smid", bufs=4) as smid,
            tc.tile_pool(name="dpool", bufs=2) as dpool,
            tc.tile_pool(name="apool", bufs=2) as apool,
            tc.tile_pool(name="prpool", bufs=10) as prpool,
            tc.tile_pool(name="fpool", bufs=2) as fpool,
            tc.tile_pool(name="opool", bufs=2) as opool,
            tc.tile_pool(name="ps_s", bufs=3, space="PSUM") as ps_s,
            tc.tile_pool(name="ps_x", bufs=3, space="PSUM") as ps_x,
            tc.tile_pool(name="ps_h", bufs=2, space="PSUM") as ps_h,
        ):
            # startup: sample-0 search inputs and the first ws tap chunks
            # land first, each on its own queue, so the first search
            # matmuls start as early as possible
            ws_t = wpool.tile([128, 9, 2, 256], BF16, tag="ws")
            bb_t = wpool.tile([128, 8], F32, tag="bb")
            id_t = wpool.tile([128, 128], BF16, tag="id")
            nc.gpsimd.dma_start(bb_t[:], bb_d[:])
            for t9 in range(9):
                eng = nc.sync if t9 % 2 == 0 else nc.gpsimd
                eng.dma_start(ws_t[:, t9:t9 + 1, :, :],
                              ws_d[:, t9:t9 + 1, :, :])
            wk_t = wpool.tile([128, 9, 2, 256], BF16, tag="wk")
            xk_t = [wpool.tile([128, NB, SK * SK], BF16, tag=f"xk{j}",
                               name=f"xk{j}") for j in range(2)]
            for tc3 in range(3):
                nc.gpsimd.dma_start(wk_t[:, 3 * tc3:3 * (tc3 + 1), :, :],
                                    wk_d[:, 3 * tc3:3 * (tc3 + 1), :, :])
            for j in range(2):
                nc.gpsimd.dma_start(
                    xk_t[j][:].rearrange("p s t -> p (s t)"), xk_d[j, :, :])
            nc.gpsimd.dma_start(id_t[:], id_d[:])
            w1_t = wpool.tile([128, 2, 256], BF16, tag="w1")
            w2_t = wpool.tile([128, 2, 256], BF16, tag="w2")
            nc.gpsimd.dma_start(w1_t[:], w1_d[:])
            nc.gpsimd.dma_start(w2_t[:], w2_d[:])

            def bias(col):
                return bb_t[:, col:col + 1]

            # ---- kernel branch, all samples at once (N = 8*5*5 = 200) ----
            kf_t = [wpool.tile([128, NB * NT], F32, tag=f"kf{m}",
                               name=f"kf{m}") for m in range(2)]

            def kernel_conv():
                for m in range(2):
                    pk = ps_h.tile([128, 512], F32, tag="ph",
                                   name=f"pk{m}")
                    first = True
                    for t in range(9):
                        ky, kx = divmod(t, 3)
                        for j in range(2):
                            rhs = xk_t[j][:].rearrange(
                                "p s (a b) -> p s a b", a=SK, b=SK)[
                                :, :, ky:ky + OK, kx:kx + OK]
                            nc.tensor.matmul(
                                pk[:, 0:NB * NT],
                                wk_t[:, t, j, m * 128:(m + 1) * 128],
                                rhs, start=first, stop=(t == 8 and j == 1))
                            first = False
                    # psum already has the kf layout [s, 5, 5] flattened
                    nc.scalar.activation(
                        kf_t[m][:, 0:NB * NT],
                        pk[:, 0:NB * NT],
                        RELU, bias=bias(2 + m))

            # ---- per-sample state ----
            ss_all = {}
            acc_all = {}
            dg_all = {}

            def split_for(s):
                sp = SPLIT_LAST if s == NB - 1 else SPLIT_MID
                n_pe, n_dve, n_act, n_pool = sp
                taps = list(range(NT))
                pe = taps[:n_pe]
                dve = taps[n_pe:n_pe + n_dve]
                act = taps[n_pe + n_dve:n_pe + n_dve + n_act]
                pool = taps[n_pe + n_dve + n_act:]
                assert len(pool) == n_pool
                return pe, dve, act, pool

            def win(ss_t, j, t):
                ky, kx = divmod(t, OK)
                return ss_t[j][:, ky:ky + OX, kx:kx + OX]

            def kcol(j, s, t):
                c = s * NT + t
                return kf_t[j][:, c:c + 1]

            class Chains:
                """Emits the xcorr MAC chains for sample s.  Act-engine
                products are handed out in slices (act_slice) so the caller
                can interleave them with the next conv's PSUM evictions."""

                def __init__(self, s):
                    self.s = s
                    pe, dve, act, pool = split_for(s)
                    self.pe = pe
                    ss_t = ss_all[s]
                    self.accs = []
                    self.act_prods = []   # (prod_tile, j, t) pending Act ops
                    self.dgs = []
                    for j in range(2):
                        accD = apool.tile([128, OX, OX], BF16, tag=f"aD{j}",
                                          name=f"aD{j}_{s}")
                        accP = apool.tile([128, OX, OX], BF16, tag=f"aP{j}",
                                          name=f"aP{j}_{s}")
                        self.accs.append((accD, accP))
                        dg = dpool.tile([128, max(len(pe), 1), 128], BF16,
                                        tag=f"dg{j}", name=f"dg{j}_{s}")
                        self.dgs.append(dg)
                    # --- Act: diagonal weights for the PE taps ---
                    for j in range(2):
                        for i, t in enumerate(pe):
                            nc.scalar.activation(
                                self.dgs[j][:, i, :], id_t[:], COPY,
                                scale=kcol(j, s, t))
                    # --- DVE: chain inits + all products for Pool taps ---
                    pool_prods = [[], []]
                    for j in range(2):
                        accD, accP = self.accs[j]
                        # init accD with the first DVE tap's product
                        nc.vector.tensor_scalar(
                            accD[:], win(ss_t, j, dve[0]),
                            kcol(j, s, dve[0]), None, MULT)
                        # init accP with the first Pool tap's product
                        nc.vector.tensor_scalar(
                            accP[:], win(ss_t, j, pool[0]),
                            kcol(j, s, pool[0]), None, MULT)
                        for t in pool[1:]:
                            pr = prpool.tile([128, OX, OX], BF16,
                                             tag=f"pp{j}",
                                             name=f"pp{j}_{s}_{t}")
                            nc.vector.tensor_scalar(
                                pr[:], win(ss_t, j, t),
                                kcol(j, s, t), None, MULT)
                            pool_prods[j].append(pr)
                    # --- Pool: the adds consuming those products ---
                    for j in range(2):
                        _, accP = self.accs[j]
                        for pr in pool_prods[j]:
                            nc.gpsimd.tensor_tensor(accP[:], accP[:], pr[:],
                                                    ADD)
                    # --- DVE: own product+add taps ---
                    for j in range(2):
                        accD, _ = self.accs[j]
                        for t in dve[1:]:
                            pr = prpool.tile([128, OX, OX], BF16,
                                             tag=f"pd{j}",
                                             name=f"pd{j}_{s}_{t}")
                            nc.vector.tensor_scalar(
                                pr[:], win(ss_t, j, t),
                                kcol(j, s, t), None, MULT)
                            nc.vector.tensor_tensor(accD[:], accD[:], pr[:],
                                                    ADD)
                    # queue up Act-product taps; adds emitted in finish()
                    for j in range(2):
                        for t in act:
                            self.act_prods.append((j, t))
                    self._act_emitted = 0
                    self._adds = []

                def act_slice(self, frac):
                    """Emit Act products up to the given fraction of the
                    total, recording the matching DVE adds for finish()."""
                    ss_t = ss_all[self.s]
                    upto = int(round(frac * len(self.act_prods)))
                    while self._act_emitted < upto:
                        j, t = self.act_prods[self._act_emitted]
                        pr = prpool.tile([128, OX, OX], BF16, tag=f"pa{j}",
                                         name=f"pa{j}_{self.s}_{t}")
                        nc.scalar.activation(pr[:], win(ss_t, j, t), COPY,
                                             scale=kcol(j, self.s, t))
                        self._adds.append((j, pr))
                        self._act_emitted += 1

                def finish(self):
                    self.act_slice(1.0)
                    # DVE adds for the Act products, then the final merge
                    for j, pr in self._adds:
                        accD, _ = self.accs[j]
                        nc.vector.tensor_tensor(accD[:], accD[:], pr[:], ADD)
                    for j in range(2):
                        accD, accP = self.accs[j]
                        nc.vector.tensor_tensor(accD[:], accD[:], accP[:],
                                                ADD)
                    acc_all[self.s] = [a for a, _ in self.accs]
                    dg_all[self.s] = self.dgs

            def search_conv(s, chains=None):
                xs_t = [xin.tile([128, SS, SS], BF16, tag=f"xs{j}",
                                 name=f"xs{j}_{s}") for j in range(2)]
                for j in range(2):
                    nc.sync.dma_start(
                        xs_t[j][:, 0:16, :],
                        xs_d[s, j * 128:(j + 1) * 128, 0:16, :])
                for j in range(2):
                    nc.sync.dma_start(
                        xs_t[j][:, 16:SS, :],
                        xs_d[s, j * 128:(j + 1) * 128, 16:SS, :])
                ss_t = [smid.tile([128, OS, OS], BF16, tag=f"ss{m}",
                                  name=f"ss{m}_{s}") for m in range(2)]
                k = 0
                for ci, (r0, nr) in enumerate(S_CHUNKS):
                    for m in range(2):
                        psm = ps_s.tile([128, 512], F32, tag="ps",
                                        name=f"psm{m}_{ci}_{s}")
                        first = True
                        for t in range(9):
                            ky, kx = divmod(t, 3)
                            for j in range(2):
                                rhs = xs_t[j][:, r0 + ky:r0 + ky + nr,
                                              kx:kx + OS]
                                nc.tensor.matmul(
                                    psm[:, 0:nr * OS],
                                    ws_t[:, t, j, m * 128:(m + 1) * 128],
                                    rhs, start=first,
                                    stop=(t == 8 and j == 1))
                                first = False
                        nc.scalar.activation(
                            ss_t[m][:, r0:r0 + nr, :],
                            psm[:, 0:nr * OS],
                            RELU, bias=bias(m))
                        k += 1
                        if chains is not None:
                            chains.act_slice(k / 4.0)
                ss_all[s] = ss_t

            def xcorr_finish(s):
                """PE taps + identity-matmul fold of the chain accumulator,
                scalar-engine eviction, then the two 1x1 heads."""
                pe_taps, _, _, _ = split_for(s)
                ss_t = ss_all.pop(s)
                accs = acc_all.pop(s)
                dgs = dg_all.pop(s)

                ft_t = [fpool.tile([128, OX, OX], BF16, tag=f"ft{j}",
                                   name=f"ft{j}_{s}") for j in range(2)]
                for j in range(2):
                    accD = accs[j]
                    dg = dgs[j]
                    for ci, (r0, nr) in enumerate(X_CHUNKS):
                        psx = ps_x.tile([128, 512], F32, tag="px",
                                        name=f"psx{j}_{ci}_{s}")
                        for i, t in enumerate(pe_taps):
                            ky, kx = divmod(t, OK)
                            rhs = ss_t[j][:, r0 + ky:r0 + ky + nr,
                                          kx:kx + OX]
                            nc.tensor.matmul(
                                psx[:, 0:nr * OX], dg[:, i, :],
                                rhs, start=(i == 0), stop=False)
                        # fold the merged DVE/Pool accumulator into PSUM
                        nc.tensor.matmul(
                            psx[:, 0:nr * OX], id_t[:],
                            accD[:, r0:r0 + nr, :],
                            start=(len(pe_taps) == 0), stop=True)
                        nc.scalar.activation(
                            ft_t[j][:, r0:r0 + nr, :],
                            psx[:, 0:nr * OX], COPY)

                # 1x1 heads
                f2_t = [fpool.tile([128, OX, OX], BF16, tag=f"f2{m}",
                                   name=f"f2{m}_{s}") for m in range(2)]
                for m in range(2):
                    for ci, (r0, nr) in enumerate(X_CHUNKS):
                        ps1 = ps_h.tile([128, 512], F32, tag="ph",
                                        name=f"ps1{m}_{ci}_{s}")
                        for j in range(2):
                            nc.tensor.matmul(
                                ps1[:, 0:nr * OX],
                                w1_t[:, j, m * 128:(m + 1) * 128],
                                ft_t[j][:, r0:r0 + nr, :],
                                start=(j == 0), stop=(j == 1))
                        nc.scalar.activation(
                            f2_t[m][:, r0:r0 + nr, :],
                            ps1[:, 0:nr * OX],
                            RELU, bias=bias(4 + m))

                for m in range(2):
                    out_t = opool.tile([128, NPIX], F32, tag=f"o{m}",
                                       name=f"o{m}_{s}")
                    for ci, (r0, nr) in enumerate(X_CHUNKS):
                        ps2 = ps_h.tile([128, 512], F32, tag="ph",
                                        name=f"ps2{m}_{ci}_{s}")
                        for j in range(2):
                            nc.tensor.matmul(
                                ps2[:, 0:nr * OX],
                                w2_t[:, j, m * 128:(m + 1) * 128],
                                f2_t[j][:, r0:r0 + nr, :],
                                start=(j == 0), stop=(j == 1))
                        nc.scalar.activation(
                            out_t[:, r0 * OX:(r0 + nr) * OX],
                            ps2[:, 0:nr * OX],
                            IDENT, bias=bias(6 + m))
                        nc.sync.dma_start(
                            y_d[s, m * 128:(m + 1) * 128,
                                r0 * OX:(r0 + nr) * OX],
                            out_t[:, r0 * OX:(r0 + nr) * OX])

            search_conv(0)
            kernel_conv()
            for s in range(1, NB):
                chains = Chains(s - 1)
                search_conv(s, chains)
                chains.finish()
                xcorr_finish(s - 1)
            chains = Chains(NB - 1)
            chains.finish()
            xcorr_finish(NB - 1)

    nc.compile()
    return nc


def _get_nc():
    if "nc" not in _CACHED:
        _CACHED["nc"] = _build_nc()
    return _CACHED["nc"]


def _fold_bn(w, g, b, m, v):
    scale = g / np.sqrt(v + EPS)
    return w * scale[:, None, None, None], (b - m * scale)


def _pack3x3(w):
    t = w.transpose(2, 3, 1, 0).reshape(9, 2, 128, 256)  # t, j, p, c
    return np.ascontiguousarray(
        t.transpose(2, 0, 1, 3).astype(BF16NP))


def _pack1x1(w):
    t = w[:, :, 0, 0].T.reshape(2, 128, 256)             # j, p, c
    return np.ascontiguousarray(t.transpose(1, 0, 2).astype(BF16NP))


def _make_in_maps(kernel, search, wk, gk, bk, mk, vk, ws, gs, bs, ms, vs,
                  w1, g1, b1, m1, v1, w2, b2):
    wk_f, bk_f = _fold_bn(np.asarray(wk), gk, bk, mk, vk)
    ws_f, bs_f = _fold_bn(np.asarray(ws), gs, bs, ms, vs)
    w1_f, b1_f = _fold_bn(np.asarray(w1), g1, b1, m1, v1)

    xs = np.asarray(search).astype(BF16NP)               # (B, CIN, 31, 31)
    xkp = np.asarray(kernel).astype(BF16NP).reshape(B, CIN, SK * SK)

    # bias columns: [bs0, bs1, bk0, bk1, b10, b11, b20, b21]
    bb = np.stack([bs_f[:128], bs_f[128:], bk_f[:128], bk_f[128:],
                   b1_f[:128], b1_f[128:],
                   np.asarray(b2)[:128], np.asarray(b2)[128:]],
                  axis=1).astype(np.float32)

    common = dict(
        ws=_pack3x3(ws_f), wk=_pack3x3(wk_f),
        w1=_pack1x1(w1_f), w2=_pack1x1(np.asarray(w2)),
        bb=np.ascontiguousarray(bb),
        id128=np.eye(128, dtype=BF16NP),
    )
    in_maps = []
    for c in range(NCORES):
        sl = slice(c * NB, (c + 1) * NB)
        xk_core = xkp[sl].reshape(NB, 2, 128, SK * SK)
        xk_core = np.ascontiguousarray(
            xk_core.transpose(1, 2, 0, 3).reshape(2, 128, NB * SK * SK))
        in_maps.append(dict(xs=np.ascontiguousarray(xs[sl]),
                            xk=xk_core, **common))
    return in_maps


def kernel(**inputs):
    in_maps = _make_in_maps(**inputs)
    nc = _get_nc()
    res = run_bass_kernel_spmd(nc, in_maps, core_ids=list(range(NCORES)))
    out = np.concatenate([r["y"] for r in res.results], axis=0)
    return out.reshape(B, COUT, OX, OX).astype(np.float32)
